# revision 18
# baseline (speedup 1.0000x reference)
"""L21 norm kernel for Trainium2 (Bass/Tile), 8-core SPMD.

Computes sum_j sqrt(sum_i S[i,j]^2) for S of shape [8192, 16384] fp32.

Sharding: S is split along columns into 8 shards of [8192, 2048] (one per
NeuronCore). Each core computes the per-column L2 norms of its 2048
columns and DMAs the [1, 2048] norm vector out; the host sums all norms
in float64.

Per-core dataflow (memory-bound; 64 MiB HBM read per core; mid-stream
DMA runs at the ~431 GB/s SBUF-AXI fabric ceiling, so all wins are at
the stream edges):
  - The 8192 rows stream as 64 row-slices of [128 partitions, 1 row,
    2048 cols] fp32 (1 MiB HWDGE DMAs, 8 KiB contiguous DRAM per
    partition). Slice granularity keeps the ACT engine's backlog after
    the last byte bounded by one ~2 us square (a monolithic 4-row tile
    square is 7 us) and keeps PE matmul bursts close enough together
    that the HAM clock gate has no ~3.4 us idle window to re-throttle
    into.
  - ACT squares each slice (bf16 out, the PE dtype cast). Even slices
    reduce on PE (ones[128,1]^T @ sq accumulating into a [1, 2048] fp32
    PSUM row, 4 matmuls of N=512); odd slices accumulate on DVE into a
    bf16 [128, 2048] accumulator (chains restarted every ~10 adds for
    accuracy, folded into PSUM at slices 19/39 and pre-folded after
    slice 57, well before the stream ends).
  - Slices 58-61 all go to PE: a ~10 us contiguous matmul run that
    flips the HAM clock gate to full rate right before the finish.
  - Finish (slices 62-63): DMA'd as column quarters in block-major
    order (A0 B0 A1 B1 ...), so column block b is complete as early as
    possible. A-quarters square on ACT, B-quarters on DVE (tensor_mul)
    so neither engine serializes; each block's B-matmul carries the
    stop flag. The final B-quarter streams as two [128, 256] halves
    with per-half stop-matmuls (stop is scheduler metadata, so two
    stops on disjoint ranges of one PSUM bank are fine), leaving a
    post-last-byte chain of one [128,256] DVE mul -> one N=256 matmul
    -> one [1,256] sqrt -> 1 KiB DMA (~3.6 us).
  - Epilogue: sqrt chunks are emitted after all ACT squares (ACT is
    strict FIFO; a sqrt queued ahead of a later square would block it).
    norms[:, :1792] ships on the sync ring while the last sqrt runs;
    the final 1 KiB leaves on the ACT engine's own HWDGE ring right
    behind its sqrt (no cross-engine semaphore hop). Host sums the
    [1, 2048] norms in f64.
"""

import numpy as np

# Full problem shape (hardcoded per the harness contract).
R = 8192          # rows
C_FULL = 16384    # columns
N_CORES = 8
C = C_FULL // N_CORES  # 2048 columns per core
P = 128           # SBUF partitions
NBLK = 512        # matmul moving free dim (one PSUM bank of fp32)
NBLOCKS = C // NBLK

NS = R // P       # 64 row-slices of [P, 1, C]
NFULL = NS - 2    # slices 0..61 full-width; 62/63 stream as column quarters
PE_RUN = 58       # slices >= PE_RUN all go to PE (HAM warm-up run)
# DVE-accumulator restart slices (tensor_copy) and fold slices
# (pe_reduce(acc) right after that slice's add).
RESETS = (1, 21, 41)
FOLDS = (19, 39, 57)

_cached = None


def _build():
    """Build + schedule the per-core Bass program. Returns the Bacc object."""
    import concourse.bacc as bacc
    import concourse.tile as tile
    from concourse import mybir

    nc = bacc.Bacc(
        "TRN2",
        target_bir_lowering=False,
        debug=False,
        enable_asserts=False,
        num_devices=N_CORES,
        # The kernel never reads its device id (the host slices inputs and
        # sums partials), so skip the partition-id plumbing: its preamble
        # TENSOR_LOAD costs 1-3.6 us on every engine before the first DMA.
        enable_partition_id=False,
    )

    s_dram = nc.dram_tensor("S", [R, C], mybir.dt.float32, kind="ExternalInput")
    out_dram = nc.dram_tensor("out", [1, C], mybir.dt.float32, kind="ExternalOutput")

    s_ap = s_dram.ap()
    out_ap = out_dram.ap()

    # [NS, P, C]: slice s, partition p = row 128*s + p -> 8 KiB contiguous
    # DRAM per (s, p) descriptor.
    v1 = s_ap.rearrange("(s p) c -> s p c", p=P)

    with tile.TileContext(nc) as tc:
        with (
            tc.tile_pool(name="io", bufs=8) as io_pool,
            tc.tile_pool(name="sqp", bufs=4) as sq_pool,
            tc.tile_pool(name="absq", bufs=8) as absq_pool,
            tc.tile_pool(name="const", bufs=1) as const_pool,
            tc.tile_pool(name="ps", bufs=1, space="PSUM") as ps_pool,
            tc.tile_pool(name="fin", bufs=1) as fin_pool,
        ):
            # First input DMA before any const setup so streaming starts as
            # early as possible. Issued from the ACT engine's HWDGE ring:
            # its preamble clears earlier than Sync's.
            x0 = io_pool.tile([P, 1, C], mybir.dt.float32, tag="x")
            nc.scalar.dma_start(out=x0[:, 0, :], in_=v1[0])

            ones = const_pool.tile([P, 1], mybir.dt.bfloat16)
            nc.vector.memset(ones, 1.0)

            # DVE-side accumulator for odd slices.
            acc = const_pool.tile([P, C], mybir.dt.bfloat16)

            # Per-column sum of squares (4 PSUM banks).
            colsq = ps_pool.tile([1, C], mybir.dt.float32)

            # Dummy sqrt: pulls the sqrt ACT-table load out of the tail.
            warm = const_pool.tile([1, 1], mybir.dt.float32)
            nc.scalar.sqrt(out=warm, in_=ones[0:1, :])

            def pe_reduce(src, first=False, blocks=range(NBLOCKS), stop_blocks=()):
                for b in blocks:
                    nc.tensor.matmul(
                        colsq[:, b * NBLK : (b + 1) * NBLK],
                        ones,
                        src[:, b * NBLK : (b + 1) * NBLK],
                        start=first,
                        stop=(b in stop_blocks),
                    )

            for s in range(NFULL):
                if s == 0:
                    xs = x0
                else:
                    xs = io_pool.tile([P, 1, C], mybir.dt.float32, tag="x")
                    nc.sync.dma_start(out=xs[:, 0, :], in_=v1[s])

                sq = sq_pool.tile([P, 1, C], mybir.dt.bfloat16, tag="sq")
                nc.scalar.square(out=sq, in_=xs)

                if s % 2 == 0 or s >= PE_RUN:
                    pe_reduce(sq[:, 0, :], first=(s == 0))
                elif s in RESETS:
                    nc.vector.tensor_copy(acc, sq[:, 0, :])
                else:
                    nc.vector.tensor_add(acc, acc, sq[:, 0, :])
                    if s in FOLDS:
                        pe_reduce(acc)

            # Finish: slices 62 (A) and 63 (B) as column quarters in
            # block-major DMA order. A squares on ACT, B on DVE; B's
            # matmul carries the block's stop flag.
            xa = io_pool.tile([P, 1, C], mybir.dt.float32, tag="x")
            xb = io_pool.tile([P, 1, C], mybir.dt.float32, tag="x")
            a_sq = [
                absq_pool.tile([P, NBLK], mybir.dt.bfloat16, tag="ab", name=f"a_sq{b}")
                for b in range(NBLOCKS)
            ]
            b_sq = [
                absq_pool.tile([P, NBLK], mybir.dt.bfloat16, tag="ab", name=f"b_sq{b}")
                for b in range(NBLOCKS)
            ]
            # Final B-quarter splits asymmetrically: a 384-col piece that
            # lands off the critical path, then a last [128, 128] sliver
            # whose mul -> N=128 matmul -> [1,128] sqrt -> 512 B DMA chain
            # is the shortest the hardware allows (512 B descriptors are
            # still at DMA line rate).
            HB = 3 * NBLK // 4

            for b in range(NBLOCKS):
                cols = slice(b * NBLK, (b + 1) * NBLK)
                last = b == NBLOCKS - 1
                nc.sync.dma_start(out=xa[:, 0, cols], in_=v1[NS - 2][:, cols])
                if not last:
                    nc.sync.dma_start(out=xb[:, 0, cols], in_=v1[NS - 1][:, cols])
                else:
                    # The very last bytes stream as two halves so the
                    # post-last-byte chain is a [128,256] mul -> N=256
                    # matmul -> [1,256] sqrt -> 1 KiB DMA.
                    nc.sync.dma_start(
                        out=xb[:, 0, b * NBLK : b * NBLK + HB],
                        in_=v1[NS - 1][:, b * NBLK : b * NBLK + HB],
                    )
                    nc.sync.dma_start(
                        out=xb[:, 0, b * NBLK + HB :],
                        in_=v1[NS - 1][:, b * NBLK + HB :],
                    )
                nc.scalar.square(out=a_sq[b], in_=xa[:, 0, cols])
                nc.tensor.matmul(colsq[:, cols], ones, a_sq[b], start=False, stop=False)
                if not last:
                    nc.vector.tensor_mul(b_sq[b], xb[:, 0, cols], xb[:, 0, cols])
                    nc.tensor.matmul(
                        colsq[:, cols], ones, b_sq[b], start=False, stop=True
                    )
                else:
                    for lo, hi in ((0, HB), (HB, NBLK)):
                        hcols = slice(b * NBLK + lo, b * NBLK + hi)
                        nc.vector.tensor_mul(
                            b_sq[b][:, lo:hi],
                            xb[:, 0, hcols], xb[:, 0, hcols],
                        )
                        nc.tensor.matmul(
                            colsq[:, hcols], ones, b_sq[b][:, lo:hi],
                            start=False, stop=True,
                        )

            # Sqrt chunks emitted after all ACT squares (strict FIFO: a
            # sqrt ahead of a later square would block it); the final
            # block's sqrt splits in two so only a [1,256] chunk trails
            # the last half's stop-matmul.
            norms = fin_pool.tile([1, C], mybir.dt.float32)
            chunks = [
                slice(b * NBLK, (b + 1) * NBLK) for b in range(NBLOCKS - 1)
            ] + [slice(C - NBLK, C - NBLK + HB), slice(C - NBLK + HB, C)]
            for cols in chunks:
                nc.scalar.activation(
                    norms[:, cols], colsq[:, cols],
                    mybir.ActivationFunctionType.Sqrt,
                )

            # Ship everything but the last 1 KiB on the sync ring while the
            # final half's sqrt still runs; the last piece goes out on the
            # ACT engine's own HWDGE ring right behind its sqrt (same
            # engine -> no cross-engine semaphore hop).
            head = C - NBLK + HB
            nc.sync.dma_start(out=out_ap[:, :head], in_=norms[:, :head])
            nc.scalar.dma_start(out=out_ap[:, head:], in_=norms[:, head:])

    nc.compile()
    return nc


def _get_nc():
    global _cached
    if _cached is None:
        _cached = _build()
    return _cached


def _run(S: np.ndarray, trace: bool = False):
    from concourse import bass_utils

    assert S.shape == (R, C_FULL), S.shape
    S = np.ascontiguousarray(np.asarray(S, dtype=np.float32))

    nc = _get_nc()
    in_maps = [
        {"S": np.ascontiguousarray(S[:, i * C : (i + 1) * C])} for i in range(N_CORES)
    ]
    try:
        res = bass_utils.run_bass_kernel_spmd(
            nc, in_maps, core_ids=list(range(N_CORES)), trace=trace
        )
    except Exception:
        # One retry: transient NRT/device hiccups (e.g. a wedged core from a
        # previous process) are recoverable on re-execution.
        res = bass_utils.run_bass_kernel_spmd(
            nc, in_maps, core_ids=list(range(N_CORES)), trace=trace
        )
    partials = np.array(
        [np.asarray(res.results[i]["out"], dtype=np.float64).sum() for i in range(N_CORES)],
        dtype=np.float64,
    )
    out = np.float32(partials.sum())
    return out, res


def kernel(S: np.ndarray) -> np.ndarray:
    out, _ = _run(S, trace=False)
    return np.asarray(out, dtype=np.float32)


def run_traced(S: np.ndarray):
    """For test.py: returns (output, BassKernelResults) with NTFF trace."""
    return _run(S, trace=True)
